# revision 1
# baseline (speedup 1.0000x reference)
"""HGNN message passing (gather + segment_sum + residual) on 8 trn2 cores.

out = x + segment_sum(x[src_idx], dst_idx, num_segments=N)

Strategy (node-sharded accumulation, no collectives):
  - dst nodes sharded across 8 cores (12500 nodes each); each core owns the
    edges targeting its node range and produces its [12500, 128] output slice.
  - Per core, nodes are processed in 100 blocks of 125. Edges of a block are
    bucketed by src//25000 (4 buckets) so src indices fit dma_gather's int16
    offset, padded to a static 768-slot capacity with -1 (the Q7 gather
    kernel trims trailing negatives, so padding costs no HBM traffic).
  - x rows for each 128-edge chunk are fetched with gpsimd dma_gather across
    the 4 SWDGE queues (4 Q7 core-pairs generating descriptors in parallel).
  - The segment-sum over a block is a sum of one-hot matmuls accumulated in
    PSUM: onehot[e, n] = (dst_local[e] == n) built by a DVE is_equal against
    an iota row; PSUM accumulates 24 chunk matmuls, then the residual row
    block of x is added and the block is written out.

All cores run one SPMD program; per-core data differences live entirely in
the input tensors (edge indices, dst values, residual slice).
"""
import os

import numpy as np

N_NODES = 100000
D = 128
N_CORES = 8
NODES_PER_CORE = N_NODES // N_CORES  # 12500
BLOCK = 125
NBLOCKS = NODES_PER_CORE // BLOCK  # 100
if os.environ.get("KERNEL_NBLOCKS"):  # debug-only scale-down
    NBLOCKS = int(os.environ["KERNEL_NBLOCKS"])
NBKT = 4
SRC_CHUNK = N_NODES // NBKT  # 25000
CAP = 768  # slots per (block, bucket); max observed count is ~720
CHUNKS_PER_BKT = CAP // 128  # 6
CH_PER_BLOCK = NBKT * CHUNKS_PER_BKT  # 24
NGATH = NBLOCKS * NBKT  # gathers per core (400)
NCH = NBLOCKS * CH_PER_BLOCK  # chunks per core (2400)
IDX_COLS = NGATH * (CAP // 16)  # 19200
STAGE_BUFS = 8

_cached = {}


def _build_program():
    from concourse import bacc, mybir, library_config
    import concourse.tile as tile

    nc = bacc.Bacc("TRN2", debug=False, num_swdge_queues=4)
    f32 = mybir.dt.float32
    x_t = nc.dram_tensor("x", [N_NODES, D], f32, kind="ExternalInput")
    xres_t = nc.dram_tensor("xres", [NBLOCKS * BLOCK, D], f32, kind="ExternalInput")
    idx_t = nc.dram_tensor("idx", [128, IDX_COLS], mybir.dt.int16, kind="ExternalInput")
    cnt_t = nc.dram_tensor("cnt", [1, NGATH], mybir.dt.int32, kind="ExternalInput")
    dstv_t = nc.dram_tensor("dstv", [128, NCH], f32, kind="ExternalInput")
    iota_t = nc.dram_tensor("iota", [128, BLOCK], f32, kind="ExternalInput")
    out_t = nc.dram_tensor("out", [NBLOCKS * BLOCK, D], f32, kind="ExternalOutput")

    with tile.TileContext(nc) as tc:
        with (
            tc.tile_pool(name="consts", bufs=1) as constp,
            tc.tile_pool(name="stage", bufs=STAGE_BUFS) as stagep,
            tc.tile_pool(name="oh", bufs=6) as ohp,
            tc.tile_pool(name="psum", bufs=2, space="PSUM") as psump,
            tc.tile_pool(name="resid", bufs=3) as residp,
            tc.tile_pool(name="osb", bufs=3) as osbp,
        ):
            nc.gpsimd.load_library(library_config.mlp)
            idx_sb = constp.tile([128, IDX_COLS], mybir.dt.int16)
            nc.sync.dma_start(idx_sb[:], idx_t[:])
            cnt_sb = constp.tile([1, NGATH], mybir.dt.int32)
            nc.sync.dma_start(cnt_sb[:], cnt_t[:])
            cnt_regs = [nc.gpsimd.alloc_register(f"cnt{k}") for k in range(NBKT)]
            dstv_sb = constp.tile([128, NCH], f32)
            nc.sync.dma_start(dstv_sb[:], dstv_t[:])
            iota_sb = constp.tile([128, BLOCK], f32)
            nc.sync.dma_start(iota_sb[:], iota_t[:])

            # zero the gather staging slots once: stale SBUF may hold NaN bit
            # patterns, and NaN * 0 would poison the PSUM accumulation
            for _ in range(STAGE_BUFS):
                stage = stagep.tile([128, CHUNKS_PER_BKT, D], f32)
                nc.vector.memset(stage[:], 0.0)

            for b in range(NBLOCKS):
                stages = []
                for k in range(NBKT):
                    g = b * NBKT + k
                    stage = stagep.tile([128, CHUNKS_PER_BKT, D], f32)
                    nc.gpsimd.reg_load(cnt_regs[k], cnt_sb[:1, g : g + 1])
                    nc.gpsimd.dma_gather(
                        stage[:],
                        x_t[k * SRC_CHUNK : (k + 1) * SRC_CHUNK, :],
                        idx_sb[:, g * (CAP // 16) : (g + 1) * (CAP // 16)],
                        CAP,
                        cnt_regs[k],
                        D,
                        queue_num=k,
                    )
                    stages.append(stage)
                psum = psump.tile([BLOCK, D], f32, space="PSUM")
                for k in range(NBKT):
                    for c in range(CHUNKS_PER_BKT):
                        ch = b * CH_PER_BLOCK + k * CHUNKS_PER_BKT + c
                        oh = ohp.tile([128, BLOCK], f32)
                        nc.vector.tensor_scalar(
                            oh[:],
                            iota_sb[:],
                            dstv_sb[:, ch : ch + 1],
                            None,
                            mybir.AluOpType.is_equal,
                        )
                        nc.tensor.matmul(
                            out=psum[:],
                            lhsT=oh[:],
                            rhs=stages[k][:, c, :],
                            start=(k == 0 and c == 0),
                            stop=(k == NBKT - 1 and c == CHUNKS_PER_BKT - 1),
                        )
                resid = residp.tile([BLOCK, D], f32)
                nc.sync.dma_start(resid[:], xres_t[b * BLOCK : (b + 1) * BLOCK, :])
                osb = osbp.tile([BLOCK, D], f32)
                nc.vector.tensor_add(osb[:], psum[:], resid[:])
                nc.sync.dma_start(out_t[b * BLOCK : (b + 1) * BLOCK, :], osb[:])

    nc.compile()
    return nc


def _preprocess(src, dst):
    """Build per-core idx / dstv SBUF images from the edge lists."""
    src = src.astype(np.int64)
    dst = dst.astype(np.int64)
    if NBLOCKS < NODES_PER_CORE // BLOCK:  # debug: drop edges past the cut
        keep = (dst % NODES_PER_CORE) // BLOCK < NBLOCKS
        src, dst = src[keep], dst[keep]
    E = src.shape[0]
    core = dst // NODES_PER_CORE
    blk = (dst % NODES_PER_CORE) // BLOCK
    dloc = (dst % NODES_PER_CORE) % BLOCK
    bkt = src // SRC_CHUNK
    sloc = src % SRC_CHUNK
    gidx = (core * NBLOCKS + blk) * NBKT + bkt  # global bucket id
    tot_bkt = N_CORES * NGATH

    order = np.argsort(gidx, kind="stable")
    gs = gidx[order]
    counts = np.bincount(gidx, minlength=tot_bkt)
    if counts.max() > CAP:
        raise ValueError(f"bucket overflow: {counts.max()} > {CAP}")
    starts = np.zeros(tot_bkt + 1, np.int64)
    np.cumsum(counts, out=starts[1:])
    within = np.arange(E, dtype=np.int64) - starts[gs]
    slot = gs * CAP + within

    idx_arr = np.full(tot_bkt * CAP, -1, np.int16)
    idx_arr[slot] = sloc[order].astype(np.int16)
    dst_arr = np.full(tot_bkt * CAP, -5.0, np.float32)
    dst_arr[slot] = dloc[order].astype(np.float32)
    cnt_arr = np.ascontiguousarray(
        counts.reshape(N_CORES, 1, NGATH).astype(np.int32)
    )

    # idx: logical slot i of a gather -> partition i%16, col i//16; tile 16->128
    idx_sb = (
        idx_arr.reshape(N_CORES, NGATH, CAP // 16, 16)
        .transpose(0, 3, 1, 2)
        .reshape(N_CORES, 16, IDX_COLS)
    )
    idx_sb = np.ascontiguousarray(np.tile(idx_sb, (1, 8, 1)))
    # dstv: slot i of chunk -> partition i%128, chunk col = g*6 + i//128
    dst_sb = np.ascontiguousarray(
        dst_arr.reshape(N_CORES, NGATH * CHUNKS_PER_BKT, 128).transpose(0, 2, 1)
    )
    return idx_sb, dst_sb, cnt_arr


def _run(x, src_idx, dst_idx, trace=False, trace_kwargs=None):
    from concourse import bass_utils

    if "nc" not in _cached:
        _cached["nc"] = _build_program()
    nc = _cached["nc"]

    x = np.ascontiguousarray(np.asarray(x, dtype=np.float32))
    idx_sb, dst_sb, cnt_arr = _preprocess(np.asarray(src_idx), np.asarray(dst_idx))
    iota = np.tile(np.arange(BLOCK, dtype=np.float32), (128, 1))
    in_maps = []
    for c in range(N_CORES):
        in_maps.append(
            {
                "x": x,
                "xres": x[c * NODES_PER_CORE : c * NODES_PER_CORE + NBLOCKS * BLOCK],
                "idx": idx_sb[c],
                "cnt": cnt_arr[c],
                "dstv": dst_sb[c],
                "iota": iota,
            }
        )
    kw = dict(trace_kwargs or {})
    res = bass_utils.run_bass_kernel_spmd(
        nc, in_maps, core_ids=list(range(N_CORES)), trace=trace, **kw
    )
    out = np.concatenate([r["out"] for r in res.results], axis=0)
    return out, res


def kernel(x, src_idx, dst_idx):
    out, _ = _run(x, src_idx, dst_idx)
    return out

